# revision 12
# baseline (speedup 1.0000x reference)
"""Distributed causal self-attention kernel for one TRN2 chip (8 NeuronCores).

Problem: B=2, T=2048, C=1024, H=16 heads, D=64. f32 in/out.

Sharding: DP=2 over batch x TP=4 over heads.
  core c -> (b = c//4, g = c%4), owns heads 4g..4g+3 of batch b.

Per-core device program (SPMD, identical graph on all 8 cores):
  A. qk^T = (x @ [W_q/8 | W_k])^T computed directly in transposed layout
     [512, 2048] via matmul(lhsT=W_qk_tile, rhs=x^T_tile)   (x^T fed from host)
  B. v = x @ W_v in natural layout [2048, 256], interleaved per head with a
     ones column -> [v_h | 1] so the attention AV matmul also produces the
     softmax row-sums.
  C. per head h, per 512-token chunk j: S^T tiles = k_h^T.T @ q_h^T -> exp
     (ScalarE) -> causal mask on diagonal tiles (GpSimd affine_select)
     -> y^T accumulation [65, 512] via matmul(lhsT=[v|1], rhs=exp(S^T));
     row 64 is the softmax denominator; normalize with
     reciprocal + partition_broadcast + multiply -> y^T [256, 2048].
  D. AllGather within each 4-core group (Local DRAM bounce buffers; 4-core
     groups support neither AllToAll/mesh nor Shared outputs) producing the
     full-channel y^T [1024, 2048] for this batch on every core. The read
     back is rank-independent, as SPMD requires.
  E. out[:, n_slice] = y_full^T.T @ W_proj[:, n_slice] with the output-
     channel slice host-sharded per core -> [2048, 256], DMA out.

Host: shards inputs (x transposed per batch, W_attn column-sliced with the
softmax scale folded into W_q, W_proj column-sliced), reassembles the 8
[2048, 256] output column-slices, adds b_proj.

All matmuls run as float32r (full-rate fp32 path on the PE).
"""

import numpy as np

import concourse.bass as bass
import concourse.bacc as bacc
import concourse.mybir as mybir
import concourse.tile as tile
from concourse import bass_utils

F32 = mybir.dt.float32
F32R = mybir.dt.float32r

B, T, C = 2, 2048, 1024
H, D = 16, 64
DP, TP = 2, 4
HPC = H // TP            # 4 heads per core
CH = HPC * D             # 256 channels per core
TS = T // TP             # 512-token slice per core
NCORES = DP * TP

REPLICA_GROUPS = [[0, 1, 2, 3], [4, 5, 6, 7]]


def build_kernel(trace_sim: bool = False):
    nc = bacc.Bacc("TRN2", target_bir_lowering=False, debug=False,
                   num_devices=NCORES)

    x_t = nc.dram_tensor("x_t", [C, T], F32R, kind="ExternalInput").ap()
    w_qk = nc.dram_tensor("w_qk", [C, 2 * CH], F32R, kind="ExternalInput").ap()
    b_qk = nc.dram_tensor("b_qk", [2 * CH], F32, kind="ExternalInput").ap()
    w_v = nc.dram_tensor("w_v", [C, CH], F32R, kind="ExternalInput").ap()
    b_v = nc.dram_tensor("b_v", [CH], F32, kind="ExternalInput").ap()
    w_p = nc.dram_tensor("w_p", [C, CH], F32R, kind="ExternalInput").ap()
    out = nc.dram_tensor("out", [T, CH], F32, kind="ExternalOutput").ap()

    KT = C // 128        # 8 contraction tiles
    NTT = T // 128       # 16 token tiles
    NTC = T // 512       # 4 token chunks

    from contextlib import ExitStack
    with tile.TileContext(nc, trace_sim=trace_sim) as tc, ExitStack() as ctx:
        const = ctx.enter_context(tc.tile_pool(name="const", bufs=1))
        qkp = ctx.enter_context(tc.tile_pool(name="qkp", bufs=1))
        vp = ctx.enter_context(tc.tile_pool(name="vp", bufs=1))
        yp = ctx.enter_context(tc.tile_pool(name="yp", bufs=1))
        ep = ctx.enter_context(tc.tile_pool(name="ep", bufs=3))
        rp = ctx.enter_context(tc.tile_pool(name="rp", bufs=2))
        rbp = ctx.enter_context(tc.tile_pool(name="rbp", bufs=2))
        dram = ctx.enter_context(tc.tile_pool(name="dram", bufs=1, space="DRAM"))

        # ---- persistent SBUF tensors + loads -------------------------------
        Wqk = [const.tile([128, 2 * CH], F32R, name=f"wqk{k}") for k in range(KT)]
        Wv = [const.tile([128, CH], F32R, name=f"wv{k}") for k in range(KT)]
        bqk = const.tile([128, 4], F32, name="bqk")
        bvrow = const.tile([1, CH], F32, name="bvrow")
        bvbc = const.tile([128, CH], F32, name="bvbc")
        ones4 = const.tile([128, 4], F32, name="ones4")
        nc.vector.memset(ones4[:], 1.0)

        for k in range(KT):
            nc.sync.dma_start(Wqk[k][:], w_qk[128 * k:128 * k + 128, :])
            nc.sync.dma_start(Wv[k][:], w_v[128 * k:128 * k + 128, :])
        nc.sync.dma_start(bqk[:], b_qk.rearrange("(i p) -> p i", p=128))
        nc.sync.dma_start(bvrow[:], b_v.unsqueeze(0))
        nc.gpsimd.partition_broadcast(bvbc[:], bvrow[:])

        # qk^T tiles: [o-tile i][t-chunk j] -> [128, 512]
        # i = 0,1: q rows (pre-scaled by 1/sqrt(D) on host); i = 2,3: k rows
        qkT = [[qkp.tile([128, 512], F32R, name=f"qkT{i}_{j}") for j in range(NTC)]
               for i in range(4)]
        # v tiles, head-interleaved with a ones column: [128, 4*65]
        v_sb = [vp.tile([128, HPC * (D + 1)], F32R, name=f"v{m}") for m in range(NTT)]
        # y^T output [256, 2048] as 2 tiles of [128, 2048]
        yT = [yp.tile([128, T], F32R, name=f"yT{i}") for i in range(2)]

        with tc.tile_pool(name="xp", bufs=1) as xp, \
             tc.tile_pool(name="psA", bufs=3, space="PSUM") as psA, \
             tc.tile_pool(name="psB", bufs=2, space="PSUM") as psB:
            xT = [xp.tile([128, T], F32R, name=f"xT{k}") for k in range(KT)]
            for k in range(KT):
                nc.sync.dma_start(xT[k][:], x_t[128 * k:128 * k + 128, :])

            # ---- phase A: qk^T = W_qk^T @ x^T ------------------------------
            for j in range(NTC):
                for i in range(4):
                    ps = psA.tile([128, 512], F32, name="psA", tag="psA")
                    for k in range(KT):
                        nc.tensor.matmul(
                            ps[:],
                            Wqk[k][:, 128 * i:128 * i + 128],
                            xT[k][:, 512 * j:512 * j + 512],
                            start=(k == 0), stop=(k == KT - 1))
                    nc.vector.tensor_scalar_add(qkT[i][j][:], ps[:], bqk[:, i:i + 1])

            # ---- phase B: v = x @ W_v (natural layout, +bias, interleave) --
            for m in range(NTT):
                # ones columns (col 64 of each head's 65-col group)
                ones_ap = v_sb[m].rearrange("p (h x) -> p h x", x=D + 1)[:, :, D:D + 1]
                nc.vector.tensor_copy(ones_ap, ones4.rearrange("p (h x) -> p h x", x=1))
                ps = psB.tile([128, CH], F32, name="psB", tag="psB")
                for k in range(KT):
                    nc.tensor.matmul(
                        ps[:],
                        xT[k][:, 128 * m:128 * m + 128],
                        Wv[k][:],
                        start=(k == 0), stop=(k == KT - 1))
                v_ap = v_sb[m].rearrange("p (h x) -> p h x", x=D + 1)[:, :, 0:D]
                nc.vector.tensor_add(
                    v_ap,
                    ps.rearrange("p (h d) -> p h d", d=D),
                    bvbc.rearrange("p (h d) -> p h d", d=D))

        # ---- W_proj column-slice load (overlaps attention) -----------------
        wpp = ctx.enter_context(tc.tile_pool(name="wpp", bufs=1))
        Wp = [wpp.tile([128, CH], F32R, name=f"wp{k}") for k in range(KT)]
        for k in range(KT):
            nc.sync.dma_start(Wp[k][:], w_p[128 * k:128 * k + 128, :])

        # ---- phase C: attention per head ----------------------------------
        with tc.tile_pool(name="psS", bufs=3, space="PSUM") as psS, \
             tc.tile_pool(name="psY", bufs=2, space="PSUM") as psY:
            for h in range(HPC):
                qi, qb = h // 2, 64 * (h % 2)
                for j in range(NTC):
                    y_ps = psY.tile([D + 1, 512], F32, name="y_ps", tag="y_ps")
                    n_s = 4 * (j + 1)           # causal s-tiles for this chunk
                    for sp in range(n_s // 2):  # pairs of 128-row s-tiles
                        s_ps = psS.tile([128, 1024], F32, name="s_ps", tag="s_ps")
                        e_sb = ep.tile([128, 1024], F32R, name="e_sb", tag="e_sb")
                        for half in range(2):
                            st = 2 * sp + half
                            nc.tensor.matmul(
                                s_ps[:, 512 * half:512 * half + 512],
                                qkT[2 + qi][st // 4][qb:qb + 64,
                                                        128 * (st % 4):128 * (st % 4) + 128],
                                qkT[qi][j][qb:qb + 64, :],
                                start=True, stop=True)
                        nc.scalar.activation(
                            e_sb[:], s_ps[:], mybir.ActivationFunctionType.Exp)
                        if 2 * sp >= 4 * j:     # pair straddles the diagonal
                            r_idx = (2 * sp - 4 * j) // 2
                            nc.gpsimd.affine_select(
                                out=e_sb[:], in_=e_sb[:],
                                compare_op=mybir.AluOpType.is_ge,
                                fill=0.0,
                                base=-256 * r_idx,
                                pattern=[[-128, 2], [1, 512]],
                                channel_multiplier=-1)
                        for half in range(2):
                            st = 2 * sp + half
                            nc.tensor.matmul(
                                y_ps[:],
                                v_sb[st][:, (D + 1) * h:(D + 1) * h + D + 1],
                                e_sb[:, 512 * half:512 * half + 512],
                                start=(st == 0), stop=(st == n_s - 1))
                    # normalize: y / rowsum
                    r_sb = rp.tile([1, 512], F32, name="r_sb", tag="r_sb")
                    nc.vector.reciprocal(r_sb[:], y_ps[D:D + 1, :])
                    rbc = rbp.tile([D, 512], F32, name="rbc", tag="rbc")
                    nc.gpsimd.partition_broadcast(rbc[:], r_sb[:])
                    nc.vector.tensor_mul(
                        yT[h // 2][64 * (h % 2):64 * (h % 2) + 64,
                                   512 * j:512 * j + 512],
                        y_ps[0:D, :], rbc[:])

        # ---- phase D: AllGather head channels within the 4-core group ------
        cc_in = dram.tile([CH, T], F32R, name="cc_in")
        cc_out = dram.tile([C, T], F32R, name="cc_out")
        for i in range(2):
            nc.sync.dma_start(cc_in[128 * i:128 * i + 128, :], yT[i][:])
        nc.gpsimd.collective_compute(
            "AllGather", mybir.AluOpType.bypass,
            replica_groups=REPLICA_GROUPS,
            ins=[cc_in.opt()], outs=[cc_out.opt()])

        # ---- phase E: out[:, n_slice] = y_full^T.T @ W_proj[:, n_slice] ----
        with tc.tile_pool(name="yf", bufs=1) as yfp, \
             tc.tile_pool(name="osb", bufs=3) as osb, \
             tc.tile_pool(name="psE", bufs=4, space="PSUM") as psE:
            yf = [yfp.tile([128, T], F32R, name=f"yf{k}") for k in range(KT)]
            for k in range(KT):
                nc.sync.dma_start(yf[k][:], cc_out[128 * k:128 * k + 128, :])
            for m in range(NTT):
                o_sb = osb.tile([128, CH], F32, name="o_sb", tag="o_sb")
                ps = psE.tile([128, CH], F32, name="psE", tag="psE")
                for k in range(KT):
                    nc.tensor.matmul(
                        ps[:],
                        yf[k][:, 128 * m:128 * m + 128],
                        Wp[k][:],
                        start=(k == 0), stop=(k == KT - 1))
                nc.vector.tensor_copy(o_sb[:], ps[:])
                nc.sync.dma_start(out[128 * m:128 * m + 128, :], o_sb[:])

    nc.compile()
    return nc


def shard_inputs(x, W_attn, b_attn, W_proj, b_proj):
    scale = np.float32(D ** -0.5)
    in_maps = []
    for c in range(NCORES):
        b, g = divmod(c, TP)
        q = slice(CH * g, CH * (g + 1))
        k = slice(C + CH * g, C + CH * (g + 1))
        v = slice(2 * C + CH * g, 2 * C + CH * (g + 1))
        W_qk = np.concatenate([W_attn[:, q] * scale, W_attn[:, k]], axis=1)
        b_qk = np.concatenate([b_attn[q] * scale, b_attn[k]])
        in_maps.append({
            "x_t": np.ascontiguousarray(x[b].T, dtype=np.float32),
            "w_qk": np.ascontiguousarray(W_qk, dtype=np.float32),
            "b_qk": np.ascontiguousarray(b_qk, dtype=np.float32),
            "w_v": np.ascontiguousarray(W_attn[:, v], dtype=np.float32),
            "b_v": np.ascontiguousarray(b_attn[v], dtype=np.float32),
            "w_p": np.ascontiguousarray(W_proj[:, CH * g:CH * (g + 1)],
                                        dtype=np.float32),
        })
    return in_maps


_NC_CACHE = {}


def get_compiled():
    if "nc" not in _NC_CACHE:
        _NC_CACHE["nc"] = build_kernel()
    return _NC_CACHE["nc"]


def run_on_hw(in_maps, **kwargs):
    nc = get_compiled()
    return bass_utils.run_bass_kernel_spmd(
        nc, in_maps, core_ids=list(range(NCORES)), **kwargs)


def kernel(x, W_attn, b_attn, W_proj, b_proj):
    x = np.asarray(x, dtype=np.float32)
    W_attn = np.asarray(W_attn, dtype=np.float32)
    b_attn = np.asarray(b_attn, dtype=np.float32)
    W_proj = np.asarray(W_proj, dtype=np.float32)
    b_proj = np.asarray(b_proj, dtype=np.float32)

    in_maps = shard_inputs(x, W_attn, b_attn, W_proj, b_proj)
    res = run_on_hw(in_maps)
    out = np.zeros((B, T, C), dtype=np.float32)
    for c in range(NCORES):
        b, g = divmod(c, TP)
        out[b, :, CH * g:CH * (g + 1)] = res.results[c]["out"]
    out += b_proj[None, None, :]
    return out


# revision 15
# speedup vs baseline: 1.2325x; 1.2325x over previous
"""Distributed causal self-attention kernel for one TRN2 chip (8 NeuronCores).

Problem: B=2, T=2048, C=1024, H=16 heads, D=64. f32 in/out.

Sharding: DP=2 over batch x TP=4 over heads.
  core c -> (b = c//4, g = c%4), owns heads 4g..4g+3 of batch b.

Per-core device program (SPMD, identical graph on all 8 cores):
  A. qk^T = (x @ [W_q/8 | W_k])^T computed directly in transposed layout
     [512, 2048] via matmul(lhsT=W_qk_tile, rhs=x^T_tile); x^T is fed from
     the host, so no on-chip transposes are needed anywhere. Output cast to
     bf16 in the PSUM->SBUF evacuation (fp32 PSUM accumulation).
  B. v = x @ W_v in natural layout [2048, 256], interleaved per head with a
     ones column -> [v_h | 1] so the attention AV matmul also produces the
     softmax row-sums. bf16.
  C. token-chunk-major attention: for each 512-token chunk j, for each head:
     S^T tiles = k_h^T.T @ q_h^T (bf16, fp32 PSUM) -> exp (ScalarE, bf16 out)
     -> causal mask on diagonal tiles (GpSimd affine_select) -> y^T
     accumulation [65, 512] via matmul(lhsT=[v|1], rhs=exp(S^T)); row 64 is
     the softmax denominator; y^T = y_unnorm / rowsum via partition_broadcast
     + tensor_tensor divide, written bf16.
  D. per-chunk 8-core AllGather of the bf16 y^T chunk (256KB in, 2MB out):
     overlaps the next chunk's attention. Output rows are rank-major =
     [batch0 chans | batch1 chans]; every core reads identical offsets.
  E. per-chunk out[chunk, n_slice] = y_gath^T.T @ W_proj_padded where the
     host gives each core W_proj rows zero-padded to 2048 so the contraction
     picks out its own batch - keeping SPMD reads rank-independent.

Host: shards inputs (x transposed per batch, W_attn column-sliced with the
softmax scale folded into W_q, W_proj column-sliced + batch-zero-padded),
reassembles the 8 [2048, 256] output column-slices, adds b_proj.

Projection/QKV matmuls run as float32r; attention matmuls in bf16.
"""

import numpy as np

import concourse.bass as bass
import concourse.bacc as bacc
import concourse.mybir as mybir
import concourse.tile as tile
from concourse import bass_utils

F32 = mybir.dt.float32
F32R = mybir.dt.float32r
BF16 = mybir.dt.bfloat16

B, T, C = 2, 2048, 1024
H, D = 16, 64
DP, TP = 2, 4
HPC = H // TP            # 4 heads per core
CH = HPC * D             # 256 channels per core
NCORES = DP * TP

RG8 = [[0, 1, 2, 3, 4, 5, 6, 7]]


def build_kernel(trace_sim: bool = False):
    nc = bacc.Bacc("TRN2", target_bir_lowering=False, debug=False,
                   num_devices=NCORES)

    x_t = nc.dram_tensor("x_t", [C, T], F32R, kind="ExternalInput").ap()
    w_qk = nc.dram_tensor("w_qk", [C, 2 * CH], F32R, kind="ExternalInput").ap()
    b_qk = nc.dram_tensor("b_qk", [2 * CH], F32, kind="ExternalInput").ap()
    w_v = nc.dram_tensor("w_v", [C, CH], F32R, kind="ExternalInput").ap()
    b_v = nc.dram_tensor("b_v", [CH], F32, kind="ExternalInput").ap()
    w_p = nc.dram_tensor("w_p", [B * C, CH], F32, kind="ExternalInput").ap()
    out = nc.dram_tensor("out", [T, CH], F32, kind="ExternalOutput").ap()

    KT = C // 128        # 8 contraction tiles for C
    KT2 = B * C // 128   # 16 contraction tiles for the padded projection
    NTT = T // 128       # 16 token tiles
    NTC = T // 512       # 4 token chunks

    from contextlib import ExitStack
    with tile.TileContext(nc, trace_sim=trace_sim) as tc, ExitStack() as ctx:
        const = ctx.enter_context(tc.tile_pool(name="const", bufs=1))
        qkp = ctx.enter_context(tc.tile_pool(name="qkp", bufs=1))
        vp = ctx.enter_context(tc.tile_pool(name="vp", bufs=1))
        yp = ctx.enter_context(tc.tile_pool(name="yp", bufs=1))
        ep = ctx.enter_context(tc.tile_pool(name="ep", bufs=3))
        rp = ctx.enter_context(tc.tile_pool(name="rp", bufs=2))
        rbp = ctx.enter_context(tc.tile_pool(name="rbp", bufs=2))
        wpp = ctx.enter_context(tc.tile_pool(name="wpp", bufs=1))
        yfp = ctx.enter_context(tc.tile_pool(name="yfp", bufs=2))
        osb = ctx.enter_context(tc.tile_pool(name="osb", bufs=3))
        dram = ctx.enter_context(tc.tile_pool(name="dram", bufs=1, space="DRAM"))

        # ---- persistent SBUF tensors + loads -------------------------------
        Wqk = [const.tile([128, 2 * CH], F32R, name=f"wqk{k}") for k in range(KT)]
        Wv = [const.tile([128, CH], F32R, name=f"wv{k}") for k in range(KT)]
        bqk = const.tile([128, 4], F32, name="bqk")
        bvrow = const.tile([1, CH], F32, name="bvrow")
        bvbc = const.tile([128, CH], F32, name="bvbc")
        ones4 = const.tile([128, 4], F32, name="ones4")
        nc.vector.memset(ones4[:], 1.0)

        for k in range(KT):
            nc.sync.dma_start(Wqk[k][:], w_qk[128 * k:128 * k + 128, :])
            nc.sync.dma_start(Wv[k][:], w_v[128 * k:128 * k + 128, :])
        nc.sync.dma_start(bqk[:], b_qk.rearrange("(i p) -> p i", p=128))
        nc.sync.dma_start(bvrow[:], b_v.unsqueeze(0))
        nc.gpsimd.partition_broadcast(bvbc[:], bvrow[:])

        # W_proj (padded to 2048 rows) -> bf16 tiles, converted on device
        Wp = [wpp.tile([128, CH], BF16, name=f"wp{k}") for k in range(KT2)]
        wpf = [wpp.tile([128, CH], F32, name=f"wpf{k}") for k in range(KT2)]
        for k in range(KT2):
            nc.sync.dma_start(wpf[k][:], w_p[128 * k:128 * k + 128, :])
            nc.vector.tensor_copy(Wp[k][:], wpf[k][:])

        # qk^T tiles (bf16): [o-tile i][t-chunk j] -> [128, 512]
        # i = 0,1: q rows (pre-scaled by 1/sqrt(D) on host); i = 2,3: k rows
        qkT = [[qkp.tile([128, 512], BF16, name=f"qkT{i}_{j}") for j in range(NTC)]
               for i in range(4)]
        # v tiles (bf16), head-interleaved with a ones column: [128, 4*65]
        v_sb = [vp.tile([128, HPC * (D + 1)], BF16, name=f"v{m}") for m in range(NTT)]
        # normalized y^T chunk tiles (bf16): [chunk j] -> [256, 512] as 2x128
        yT = [[yp.tile([128, 512], BF16, name=f"yT{i}_{j}") for j in range(NTC)]
              for i in range(2)]

        # AllGather bounce buffers, one pair per chunk
        cc_in = [dram.tile([CH, 512], BF16, name=f"cc_in{j}") for j in range(NTC)]
        cc_out = [dram.tile([NCORES * CH, 512], BF16, name=f"cc_out{j}",
                            addr_space="Shared")
                  for j in range(NTC)]

        with tc.tile_pool(name="xp", bufs=1) as xp, \
             tc.tile_pool(name="psA", bufs=3, space="PSUM") as psA, \
             tc.tile_pool(name="psB", bufs=2, space="PSUM") as psB:
            # x^T loaded in 512-column chunks so phase A starts early
            xT = [[xp.tile([128, 512], F32R, name=f"xT{k}_{j}") for j in range(NTC)]
                  for k in range(KT)]
            for j in range(NTC):
                for k in range(KT):
                    nc.sync.dma_start(xT[k][j][:],
                                      x_t[128 * k:128 * k + 128,
                                          512 * j:512 * j + 512])

            # ---- phase A: qk^T = W_qk^T @ x^T ------------------------------
            for j in range(NTC):
                for i in range(4):
                    ps = psA.tile([128, 512], F32, name="psA", tag="psA")
                    for k in range(KT):
                        nc.tensor.matmul(
                            ps[:],
                            Wqk[k][:, 128 * i:128 * i + 128],
                            xT[k][j][:],
                            start=(k == 0), stop=(k == KT - 1))
                    nc.vector.tensor_scalar_add(qkT[i][j][:], ps[:], bqk[:, i:i + 1])

            # ---- phase B: v = x @ W_v (natural layout, +bias, interleave) --
            for m in range(NTT):
                ones_ap = v_sb[m].rearrange("p (h x) -> p h x", x=D + 1)[:, :, D:D + 1]
                nc.vector.tensor_copy(ones_ap, ones4.rearrange("p (h x) -> p h x", x=1))
                ps = psB.tile([128, CH], F32, name="psB", tag="psB")
                for k in range(KT):
                    nc.tensor.matmul(
                        ps[:],
                        xT[k][m // 4][:, 128 * (m % 4):128 * (m % 4) + 128],
                        Wv[k][:],
                        start=(k == 0), stop=(k == KT - 1))
                v_ap = v_sb[m].rearrange("p (h x) -> p h x", x=D + 1)[:, :, 0:D]
                nc.vector.tensor_add(
                    v_ap,
                    ps.rearrange("p (h d) -> p h d", d=D),
                    bvbc.rearrange("p (h d) -> p h d", d=D))

        # ---- phases C/D/E: chunk-major attention + pipelined AG + proj -----
        with tc.tile_pool(name="psS", bufs=2, space="PSUM") as psS, \
             tc.tile_pool(name="psY", bufs=2, space="PSUM") as psY, \
             tc.tile_pool(name="psE", bufs=2, space="PSUM") as psE:
            for j in range(NTC):
                # C: attention for all heads on token chunk j
                for h in range(HPC):
                    qi, qb = h // 2, 64 * (h % 2)
                    y_ps = psY.tile([D + 1, 512], F32, name="y_ps", tag="y_ps")
                    n_s = 4 * (j + 1)           # causal s-tiles for this chunk
                    for sp in range(n_s // 2):  # pairs of 128-row s-tiles
                        s_ps = psS.tile([128, 1024], F32, name="s_ps", tag="s_ps")
                        e_sb = ep.tile([128, 1024], BF16, name="e_sb", tag="e_sb")
                        for half in range(2):
                            st = 2 * sp + half
                            nc.tensor.matmul(
                                s_ps[:, 512 * half:512 * half + 512],
                                qkT[2 + qi][st // 4][qb:qb + 64,
                                                     128 * (st % 4):128 * (st % 4) + 128],
                                qkT[qi][j][qb:qb + 64, :],
                                start=True, stop=True)
                        nc.scalar.activation(
                            e_sb[:], s_ps[:], mybir.ActivationFunctionType.Exp)
                        if 2 * sp >= 4 * j:     # pair straddles the diagonal
                            r_idx = (2 * sp - 4 * j) // 2
                            nc.gpsimd.affine_select(
                                out=e_sb[:], in_=e_sb[:],
                                compare_op=mybir.AluOpType.is_ge,
                                fill=0.0,
                                base=-256 * r_idx,
                                pattern=[[-128, 2], [1, 512]],
                                channel_multiplier=-1)
                        for half in range(2):
                            st = 2 * sp + half
                            nc.tensor.matmul(
                                y_ps[:],
                                v_sb[st][:, (D + 1) * h:(D + 1) * h + D + 1],
                                e_sb[:, 512 * half:512 * half + 512],
                                start=(st == 0), stop=(st == n_s - 1))
                    # normalize: y * (1/rowsum); broadcast the raw rowsum
                    # across partitions first so the reciprocal runs at full
                    # 64-partition width (a [1,512] reciprocal is 64x slower)
                    r_sb = rp.tile([1, 512], F32, name="r_sb", tag="r_sb")
                    nc.scalar.copy(r_sb[:], y_ps[D:D + 1, :])
                    rbc = rbp.tile([D, 512], F32, name="rbc", tag="rbc")
                    rinv = rbp.tile([D, 512], F32, name="rinv", tag="rinv")
                    nc.gpsimd.partition_broadcast(rbc[:], r_sb[:])
                    nc.vector.reciprocal_approx_fast(rinv[:], rbc[:])
                    nc.vector.tensor_mul(
                        yT[h // 2][j][64 * (h % 2):64 * (h % 2) + 64, :],
                        y_ps[0:D, :], rinv[:])

                # D: ship chunk j through the 8-core AllGather
                for i in range(2):
                    nc.sync.dma_start(cc_in[j][128 * i:128 * i + 128, :],
                                      yT[i][j][:])
                nc.gpsimd.collective_compute(
                    "AllGather", mybir.AluOpType.bypass,
                    replica_groups=RG8,
                    ins=[cc_in[j].opt()], outs=[cc_out[j].opt()])

                # E: projection for chunk j (zero-padded W_proj selects batch)
                yf = [yfp.tile([128, 512], BF16, name=f"yf{k}", tag=f"yf{k}")
                      for k in range(KT2)]
                for k in range(KT2):
                    nc.sync.dma_start(yf[k][:],
                                      cc_out[j][128 * k:128 * k + 128, :])
                for m in range(4):
                    o_sb = osb.tile([128, CH], F32, name="o_sb", tag="o_sb")
                    ps = psE.tile([128, CH], F32, name="psE", tag="psE")
                    for k in range(KT2):
                        nc.tensor.matmul(
                            ps[:],
                            yf[k][:, 128 * m:128 * m + 128],
                            Wp[k][:],
                            start=(k == 0), stop=(k == KT2 - 1))
                    nc.vector.tensor_copy(o_sb[:], ps[:])
                    nc.sync.dma_start(
                        out[512 * j + 128 * m:512 * j + 128 * m + 128, :],
                        o_sb[:])

    nc.compile()
    return nc


def shard_inputs(x, W_attn, b_attn, W_proj, b_proj):
    scale = np.float32(D ** -0.5)
    in_maps = []
    for c in range(NCORES):
        b, g = divmod(c, TP)
        q = slice(CH * g, CH * (g + 1))
        k = slice(C + CH * g, C + CH * (g + 1))
        v = slice(2 * C + CH * g, 2 * C + CH * (g + 1))
        W_qk = np.concatenate([W_attn[:, q] * scale, W_attn[:, k]], axis=1)
        b_qk = np.concatenate([b_attn[q] * scale, b_attn[k]])
        # W_proj column slice, zero-padded to 2048 rows so the contraction
        # over the 8-core-gathered [2048, t] y picks out this core's batch
        w_p_pad = np.zeros((B * C, CH), dtype=np.float32)
        w_p_pad[C * b:C * (b + 1)] = W_proj[:, CH * g:CH * (g + 1)]
        in_maps.append({
            "x_t": np.ascontiguousarray(x[b].T, dtype=np.float32),
            "w_qk": np.ascontiguousarray(W_qk, dtype=np.float32),
            "b_qk": np.ascontiguousarray(b_qk, dtype=np.float32),
            "w_v": np.ascontiguousarray(W_attn[:, v], dtype=np.float32),
            "b_v": np.ascontiguousarray(b_attn[v], dtype=np.float32),
            "w_p": w_p_pad,
        })
    return in_maps


_NC_CACHE = {}


def get_compiled():
    if "nc" not in _NC_CACHE:
        _NC_CACHE["nc"] = build_kernel()
    return _NC_CACHE["nc"]


def run_on_hw(in_maps, **kwargs):
    nc = get_compiled()
    return bass_utils.run_bass_kernel_spmd(
        nc, in_maps, core_ids=list(range(NCORES)), **kwargs)


def kernel(x, W_attn, b_attn, W_proj, b_proj):
    x = np.asarray(x, dtype=np.float32)
    W_attn = np.asarray(W_attn, dtype=np.float32)
    b_attn = np.asarray(b_attn, dtype=np.float32)
    W_proj = np.asarray(W_proj, dtype=np.float32)
    b_proj = np.asarray(b_proj, dtype=np.float32)

    in_maps = shard_inputs(x, W_attn, b_attn, W_proj, b_proj)
    res = run_on_hw(in_maps)
    out = np.zeros((B, T, C), dtype=np.float32)
    for c in range(NCORES):
        b, g = divmod(c, TP)
        out[b, :, CH * g:CH * (g + 1)] = res.results[c]["out"]
    out += b_proj[None, None, :]
    return out


# revision 17
# speedup vs baseline: 1.3414x; 1.0884x over previous
"""Distributed causal self-attention kernel for one TRN2 chip (8 NeuronCores).

Problem: B=2, T=2048, C=1024, H=16 heads, D=64. f32 in/out.

Sharding: DP=2 over batch x TP=4 over heads.
  core c -> (b = c//4, g = c%4), owns heads 4g..4g+3 of batch b.

Per-core device program (SPMD, identical graph on all 8 cores):
  A. qk^T = (x @ [W_q/8 | W_k])^T computed directly in transposed layout
     [512, 2048] via matmul(lhsT=W_qk_tile, rhs=x^T_tile); x^T is fed from
     the host, so no on-chip transposes are needed anywhere. Output cast to
     bf16 in the PSUM->SBUF evacuation (fp32 PSUM accumulation).
  B. v = x @ W_v in natural layout [2048, 256], interleaved per head with a
     ones column -> [v_h | 1] so the attention AV matmul also produces the
     softmax row-sums. bf16.
  C. token-chunk-major attention: for each 512-token chunk j, for each head:
     S^T tiles = k_h^T.T @ q_h^T (bf16, fp32 PSUM) -> exp (ScalarE, bf16 out)
     -> causal mask on diagonal tiles (GpSimd affine_select) -> y^T
     accumulation [65, 512] via matmul(lhsT=[v|1], rhs=exp(S^T)); row 64 is
     the softmax denominator; y^T = y_unnorm / rowsum via partition_broadcast
     + tensor_tensor divide, written bf16.
  D. per-chunk 8-core AllGather of the bf16 y^T chunk (256KB in, 2MB out):
     overlaps the next chunk's attention. Output rows are rank-major =
     [batch0 chans | batch1 chans]; every core reads identical offsets.
  E. per-chunk out[chunk, n_slice] = y_gath^T.T @ W_proj_padded where the
     host gives each core W_proj rows zero-padded to 2048 so the contraction
     picks out its own batch - keeping SPMD reads rank-independent.

Host: shards inputs (x transposed per batch, W_attn column-sliced with the
softmax scale folded into W_q, W_proj column-sliced + batch-zero-padded),
reassembles the 8 [2048, 256] output column-slices, adds b_proj.

Projection/QKV matmuls run as float32r; attention matmuls in bf16.
"""

import numpy as np

import concourse.bass as bass
import concourse.bacc as bacc
import concourse.mybir as mybir
import concourse.tile as tile
from concourse import bass_utils

F32 = mybir.dt.float32
F32R = mybir.dt.float32r
BF16 = mybir.dt.bfloat16

B, T, C = 2, 2048, 1024
H, D = 16, 64
DP, TP = 2, 4
HPC = H // TP            # 4 heads per core
CH = HPC * D             # 256 channels per core
NCORES = DP * TP

RG8 = [[0, 1, 2, 3, 4, 5, 6, 7]]


def build_kernel(trace_sim: bool = False):
    nc = bacc.Bacc("TRN2", target_bir_lowering=False, debug=False,
                   num_devices=NCORES)

    x_t = nc.dram_tensor("x_t", [C, T], F32R, kind="ExternalInput").ap()
    w_qk = nc.dram_tensor("w_qk", [C, 2 * CH], F32R, kind="ExternalInput").ap()
    b_qk = nc.dram_tensor("b_qk", [2 * CH], F32, kind="ExternalInput").ap()
    w_v = nc.dram_tensor("w_v", [C, CH], F32R, kind="ExternalInput").ap()
    b_v = nc.dram_tensor("b_v", [CH], F32, kind="ExternalInput").ap()
    w_p = nc.dram_tensor("w_p", [B * C, CH], F32, kind="ExternalInput").ap()
    out = nc.dram_tensor("out", [T, CH], F32, kind="ExternalOutput").ap()

    KT = C // 128        # 8 contraction tiles for C
    KT2 = B * C // 128   # 16 contraction tiles for the padded projection
    NTT = T // 128       # 16 token tiles
    NTC = T // 512       # 4 token chunks

    from contextlib import ExitStack
    with tile.TileContext(nc, trace_sim=trace_sim) as tc, ExitStack() as ctx:
        const = ctx.enter_context(tc.tile_pool(name="const", bufs=1))
        qkp = ctx.enter_context(tc.tile_pool(name="qkp", bufs=1))
        vp = ctx.enter_context(tc.tile_pool(name="vp", bufs=1))
        yp = ctx.enter_context(tc.tile_pool(name="yp", bufs=1))
        ep = ctx.enter_context(tc.tile_pool(name="ep", bufs=3))
        rp = ctx.enter_context(tc.tile_pool(name="rp", bufs=2))
        rbp = ctx.enter_context(tc.tile_pool(name="rbp", bufs=2))
        wpp = ctx.enter_context(tc.tile_pool(name="wpp", bufs=1))
        yfp = ctx.enter_context(tc.tile_pool(name="yfp", bufs=2))
        osb = ctx.enter_context(tc.tile_pool(name="osb", bufs=3))
        dram = ctx.enter_context(tc.tile_pool(name="dram", bufs=1, space="DRAM"))

        # ---- persistent SBUF tensors + loads -------------------------------
        Wqk = [const.tile([128, 2 * CH], F32R, name=f"wqk{k}") for k in range(KT)]
        Wv = [const.tile([128, CH], F32R, name=f"wv{k}") for k in range(KT)]
        bqk = const.tile([128, 4], F32, name="bqk")
        bvrow = const.tile([1, CH], F32, name="bvrow")
        bvbc = const.tile([128, CH], F32, name="bvbc")
        ones4 = const.tile([128, 4], F32, name="ones4")
        nc.vector.memset(ones4[:], 1.0)

        for k in range(KT):
            nc.sync.dma_start(Wqk[k][:], w_qk[128 * k:128 * k + 128, :])
        nc.sync.dma_start(bqk[:], b_qk.rearrange("(i p) -> p i", p=128))
        nc.sync.dma_start(bvrow[:], b_v.unsqueeze(0))
        nc.gpsimd.partition_broadcast(bvbc[:], bvrow[:])

        # W_proj (padded to 2048 rows) -> bf16 tiles, converted on device.
        # (tiles declared here; DMAs emitted after the x loads below so the
        # first QKV matmuls aren't starved behind 2MB of projection weights)
        Wp = [wpp.tile([128, CH], BF16, name=f"wp{k}") for k in range(KT2)]
        wpf = [wpp.tile([128, CH], F32, name=f"wpf{k}") for k in range(KT2)]

        # qk^T tiles (bf16): [o-tile i][t-chunk j] -> [128, 512]
        # i = 0,1: q rows (pre-scaled by 1/sqrt(D) on host); i = 2,3: k rows
        qkT = [[qkp.tile([128, 512], BF16, name=f"qkT{i}_{j}") for j in range(NTC)]
               for i in range(4)]
        # v tiles (bf16), head-interleaved with a ones column: [128, 4*65]
        v_sb = [vp.tile([128, HPC * (D + 1)], BF16, name=f"v{m}") for m in range(NTT)]
        # normalized y^T chunk tiles (bf16): [chunk j] -> [256, 512] as 2x128
        yT = [[yp.tile([128, 512], BF16, name=f"yT{i}_{j}") for j in range(NTC)]
              for i in range(2)]

        # AllGather bounce buffers, one pair per chunk
        cc_in = [dram.tile([CH, 512], BF16, name=f"cc_in{j}") for j in range(NTC)]
        cc_out = [dram.tile([NCORES * CH, 512], BF16, name=f"cc_out{j}",
                            addr_space="Shared")
                  for j in range(NTC)]

        with tc.tile_pool(name="xp", bufs=1) as xp, \
             tc.tile_pool(name="psA", bufs=3, space="PSUM") as psA, \
             tc.tile_pool(name="psB", bufs=2, space="PSUM") as psB:
            # x^T loaded in 512-column chunks so phase A starts early
            xT = [[xp.tile([128, 512], F32R, name=f"xT{k}_{j}") for j in range(NTC)]
                  for k in range(KT)]
            for j in range(NTC):
                for k in range(KT):
                    nc.sync.dma_start(xT[k][j][:],
                                      x_t[128 * k:128 * k + 128,
                                          512 * j:512 * j + 512])
            for k in range(KT):
                nc.sync.dma_start(Wv[k][:], w_v[128 * k:128 * k + 128, :])
            for k in range(KT2):
                nc.sync.dma_start(wpf[k][:], w_p[128 * k:128 * k + 128, :])
                nc.vector.tensor_copy(Wp[k][:], wpf[k][:])

            # ---- phase A: qk^T = W_qk^T @ x^T ------------------------------
            for j in range(NTC):
                for i in range(4):
                    ps = psA.tile([128, 512], F32, name="psA", tag="psA")
                    for k in range(KT):
                        nc.tensor.matmul(
                            ps[:],
                            Wqk[k][:, 128 * i:128 * i + 128],
                            xT[k][j][:],
                            start=(k == 0), stop=(k == KT - 1))
                    nc.vector.tensor_scalar_add(qkT[i][j][:], ps[:], bqk[:, i:i + 1])

            # ---- phase B: v = x @ W_v (natural layout, +bias, interleave) --
            for m in range(NTT):
                ones_ap = v_sb[m].rearrange("p (h x) -> p h x", x=D + 1)[:, :, D:D + 1]
                nc.vector.tensor_copy(ones_ap, ones4.rearrange("p (h x) -> p h x", x=1))
                ps = psB.tile([128, CH], F32, name="psB", tag="psB")
                for k in range(KT):
                    nc.tensor.matmul(
                        ps[:],
                        xT[k][m // 4][:, 128 * (m % 4):128 * (m % 4) + 128],
                        Wv[k][:],
                        start=(k == 0), stop=(k == KT - 1))
                v_ap = v_sb[m].rearrange("p (h x) -> p h x", x=D + 1)[:, :, 0:D]
                nc.vector.tensor_add(
                    v_ap,
                    ps.rearrange("p (h d) -> p h d", d=D),
                    bvbc.rearrange("p (h d) -> p h d", d=D))

        # ---- phases C/D/E: chunk-major attention + pipelined AG + proj -----
        # Heads are processed in pairs (2hp, 2hp+1). The even head's q/k rows
        # live at partitions 0-63, the odd head's at 64-127, so interleaving
        # their S matmuls alternates PE row groups (tile_position auto-derives
        # from base_partition): the next weight load overlaps the in-flight
        # matmul and the two K=64 matmuls stream concurrently.
        def attn_chunk(j):
            for hp in range(HPC // 2):
                ha, hb = 2 * hp, 2 * hp + 1
                y_psA = psY.tile([D + 1, 512], F32, name="y_psA", tag="y_ps")
                y_psB = psY.tile([D + 1, 512], F32, name="y_psB", tag="y_ps")
                n_s = 4 * (j + 1)           # causal s-tiles for this chunk
                for sp in range(n_s // 2):  # pairs of 128-row s-tiles
                    sA = psS.tile([128, 1024], F32, name="sA", tag="s_ps")
                    sB = psS.tile([128, 1024], F32, name="sB", tag="s_ps")
                    eA = ep.tile([128, 1024], BF16, name="eA", tag="e_sb")
                    eB = ep.tile([128, 1024], BF16, name="eB", tag="e_sb")
                    for half in range(2):
                        st = 2 * sp + half
                        kt = qkT[2 + hp][st // 4]
                        qt = qkT[hp][j]
                        ks = 128 * (st % 4)
                        nc.tensor.matmul(
                            sA[:, 512 * half:512 * half + 512],
                            kt[0:64, ks:ks + 128], qt[0:64, :],
                            start=True, stop=True)
                        nc.tensor.matmul(
                            sB[:, 512 * half:512 * half + 512],
                            kt[64:128, ks:ks + 128], qt[64:128, :],
                            start=True, stop=True)
                    nc.scalar.activation(
                        eA[:], sA[:], mybir.ActivationFunctionType.Exp)
                    nc.scalar.activation(
                        eB[:], sB[:], mybir.ActivationFunctionType.Exp)
                    if 2 * sp >= 4 * j:     # pair straddles the diagonal
                        r_idx = (2 * sp - 4 * j) // 2
                        for e in (eA, eB):
                            nc.gpsimd.affine_select(
                                out=e[:], in_=e[:],
                                compare_op=mybir.AluOpType.is_ge,
                                fill=0.0,
                                base=-256 * r_idx,
                                pattern=[[-128, 2], [1, 512]],
                                channel_multiplier=-1)
                    for half in range(2):
                        st = 2 * sp + half
                        nc.tensor.matmul(
                            y_psA[:],
                            v_sb[st][:, (D + 1) * ha:(D + 1) * ha + D + 1],
                            eA[:, 512 * half:512 * half + 512],
                            start=(st == 0), stop=(st == n_s - 1))
                        nc.tensor.matmul(
                            y_psB[:],
                            v_sb[st][:, (D + 1) * hb:(D + 1) * hb + D + 1],
                            eB[:, 512 * half:512 * half + 512],
                            start=(st == 0), stop=(st == n_s - 1))
                # normalize: y * (1/rowsum); broadcast the raw rowsum across
                # partitions first so the reciprocal runs at full width
                for hh, y_ps in ((ha, y_psA), (hb, y_psB)):
                    r_sb = rp.tile([1, 512], F32, name="r_sb", tag="r_sb")
                    nc.vector.tensor_copy(r_sb[:], y_ps[D:D + 1, :])
                    rbc = rbp.tile([D, 512], F32, name="rbc", tag="rbc")
                    rinv = rbp.tile([D, 512], F32, name="rinv", tag="rinv")
                    nc.gpsimd.partition_broadcast(rbc[:], r_sb[:])
                    nc.vector.reciprocal_approx_fast(rinv[:], rbc[:])
                    nc.vector.tensor_mul(
                        yT[hp][j][64 * (hh % 2):64 * (hh % 2) + 64, :],
                        y_ps[0:D, :], rinv[:])

        def ship_chunk(j):
            for i in range(2):
                nc.sync.dma_start(cc_in[j][128 * i:128 * i + 128, :],
                                  yT[i][j][:])
            nc.gpsimd.collective_compute(
                "AllGather", mybir.AluOpType.bypass,
                replica_groups=RG8,
                ins=[cc_in[j].opt()], outs=[cc_out[j].opt()])

        def proj_chunk(j):
            yf = [yfp.tile([128, 512], BF16, name=f"yf{k}", tag=f"yf{k}")
                  for k in range(KT2)]
            for k in range(KT2):
                nc.sync.dma_start(yf[k][:],
                                  cc_out[j][128 * k:128 * k + 128, :])
            for m in range(4):
                o_sb = osb.tile([128, CH], F32, name="o_sb", tag="o_sb")
                ps = psE.tile([128, CH], F32, name="psE", tag="psE")
                for k in range(KT2):
                    nc.tensor.matmul(
                        ps[:],
                        yf[k][:, 128 * m:128 * m + 128],
                        Wp[k][:],
                        start=(k == 0), stop=(k == KT2 - 1))
                nc.vector.tensor_copy(o_sb[:], ps[:])
                nc.sync.dma_start(
                    out[512 * j + 128 * m:512 * j + 128 * m + 128, :],
                    o_sb[:])

        # Skew projection one chunk behind attention: PE executes in issue
        # order, so proj(j) is only enqueued after attn(j+1) - by then the
        # chunk-j AllGather has completed and the PE never stalls on it.
        with tc.tile_pool(name="psS", bufs=2, space="PSUM") as psS, \
             tc.tile_pool(name="psY", bufs=2, space="PSUM") as psY, \
             tc.tile_pool(name="psE", bufs=2, space="PSUM") as psE:
            for j in range(NTC):
                attn_chunk(j)
                ship_chunk(j)
                if j > 0:
                    proj_chunk(j - 1)
            proj_chunk(NTC - 1)

    nc.compile()
    return nc


def shard_inputs(x, W_attn, b_attn, W_proj, b_proj):
    scale = np.float32(D ** -0.5)
    in_maps = []
    for c in range(NCORES):
        b, g = divmod(c, TP)
        q = slice(CH * g, CH * (g + 1))
        k = slice(C + CH * g, C + CH * (g + 1))
        v = slice(2 * C + CH * g, 2 * C + CH * (g + 1))
        W_qk = np.concatenate([W_attn[:, q] * scale, W_attn[:, k]], axis=1)
        b_qk = np.concatenate([b_attn[q] * scale, b_attn[k]])
        # W_proj column slice, zero-padded to 2048 rows so the contraction
        # over the 8-core-gathered [2048, t] y picks out this core's batch
        w_p_pad = np.zeros((B * C, CH), dtype=np.float32)
        w_p_pad[C * b:C * (b + 1)] = W_proj[:, CH * g:CH * (g + 1)]
        in_maps.append({
            "x_t": np.ascontiguousarray(x[b].T, dtype=np.float32),
            "w_qk": np.ascontiguousarray(W_qk, dtype=np.float32),
            "b_qk": np.ascontiguousarray(b_qk, dtype=np.float32),
            "w_v": np.ascontiguousarray(W_attn[:, v], dtype=np.float32),
            "b_v": np.ascontiguousarray(b_attn[v], dtype=np.float32),
            "w_p": w_p_pad,
        })
    return in_maps


_NC_CACHE = {}


def get_compiled():
    if "nc" not in _NC_CACHE:
        _NC_CACHE["nc"] = build_kernel()
    return _NC_CACHE["nc"]


def run_on_hw(in_maps, **kwargs):
    nc = get_compiled()
    return bass_utils.run_bass_kernel_spmd(
        nc, in_maps, core_ids=list(range(NCORES)), **kwargs)


def kernel(x, W_attn, b_attn, W_proj, b_proj):
    x = np.asarray(x, dtype=np.float32)
    W_attn = np.asarray(W_attn, dtype=np.float32)
    b_attn = np.asarray(b_attn, dtype=np.float32)
    W_proj = np.asarray(W_proj, dtype=np.float32)
    b_proj = np.asarray(b_proj, dtype=np.float32)

    in_maps = shard_inputs(x, W_attn, b_attn, W_proj, b_proj)
    res = run_on_hw(in_maps)
    out = np.zeros((B, T, C), dtype=np.float32)
    for c in range(NCORES):
        b, g = divmod(c, TP)
        out[b, :, CH * g:CH * (g + 1)] = res.results[c]["out"]
    out += b_proj[None, None, :]
    return out


# revision 18
# speedup vs baseline: 1.3505x; 1.0067x over previous
"""Distributed causal self-attention kernel for one TRN2 chip (8 NeuronCores).

Problem: B=2, T=2048, C=1024, H=16 heads, D=64. f32 in/out.

Sharding: DP=2 over batch x TP=4 over heads.
  core c -> (b = c//4, g = c%4), owns heads 4g..4g+3 of batch b.

Per-core device program (SPMD, identical graph on all 8 cores):
  A. qk^T = (x @ [W_q/8 | W_k])^T computed directly in transposed layout
     [512, 2048] via matmul(lhsT=W_qk_tile, rhs=x^T_tile); x^T is fed from
     the host, so no on-chip transposes are needed anywhere. Output cast to
     bf16 in the PSUM->SBUF evacuation (fp32 PSUM accumulation).
  B. v = x @ W_v in natural layout [2048, 256], interleaved per head with a
     ones column -> [v_h | 1] so the attention AV matmul also produces the
     softmax row-sums. bf16.
  C. token-chunk-major attention: for each 512-token chunk j, for each head:
     S^T tiles = k_h^T.T @ q_h^T (bf16, fp32 PSUM) -> exp (ScalarE, bf16 out)
     -> causal mask on diagonal tiles (GpSimd affine_select) -> y^T
     accumulation [65, 512] via matmul(lhsT=[v|1], rhs=exp(S^T)); row 64 is
     the softmax denominator; y^T = y_unnorm / rowsum via partition_broadcast
     + tensor_tensor divide, written bf16.
  D. per-chunk 8-core AllGather of the bf16 y^T chunk (256KB in, 2MB out):
     overlaps the next chunk's attention. Output rows are rank-major =
     [batch0 chans | batch1 chans]; every core reads identical offsets.
  E. per-chunk out[chunk, n_slice] = y_gath^T.T @ W_proj_padded where the
     host gives each core W_proj rows zero-padded to 2048 so the contraction
     picks out its own batch - keeping SPMD reads rank-independent.

Host: shards inputs (x transposed per batch, W_attn column-sliced with the
softmax scale folded into W_q, W_proj column-sliced + batch-zero-padded),
reassembles the 8 [2048, 256] output column-slices, adds b_proj.

Projection/QKV matmuls run as float32r; attention matmuls in bf16.
"""

import numpy as np

import concourse.bass as bass
import concourse.bacc as bacc
import concourse.mybir as mybir
import concourse.tile as tile
from concourse import bass_utils

F32 = mybir.dt.float32
F32R = mybir.dt.float32r
BF16 = mybir.dt.bfloat16

B, T, C = 2, 2048, 1024
H, D = 16, 64
DP, TP = 2, 4
HPC = H // TP            # 4 heads per core
CH = HPC * D             # 256 channels per core
NCORES = DP * TP

RG8 = [[0, 1, 2, 3, 4, 5, 6, 7]]


def build_kernel(trace_sim: bool = False):
    nc = bacc.Bacc("TRN2", target_bir_lowering=False, debug=False,
                   num_devices=NCORES)

    x_t = nc.dram_tensor("x_t", [C, T], F32R, kind="ExternalInput").ap()
    w_qk = nc.dram_tensor("w_qk", [C, 2 * CH], F32R, kind="ExternalInput").ap()
    b_qk = nc.dram_tensor("b_qk", [2 * CH], F32, kind="ExternalInput").ap()
    w_v = nc.dram_tensor("w_v", [C, CH], F32R, kind="ExternalInput").ap()
    b_v = nc.dram_tensor("b_v", [CH], F32, kind="ExternalInput").ap()
    w_p = nc.dram_tensor("w_p", [B * C, CH], F32, kind="ExternalInput").ap()
    out = nc.dram_tensor("out", [T, CH], F32, kind="ExternalOutput").ap()

    KT = C // 128        # 8 contraction tiles for C
    KT2 = B * C // 128   # 16 contraction tiles for the padded projection
    NTT = T // 128       # 16 token tiles
    NTC = T // 512       # 4 token chunks

    from contextlib import ExitStack
    with tile.TileContext(nc, trace_sim=trace_sim) as tc, ExitStack() as ctx:
        const = ctx.enter_context(tc.tile_pool(name="const", bufs=1))
        qkp = ctx.enter_context(tc.tile_pool(name="qkp", bufs=1))
        vp = ctx.enter_context(tc.tile_pool(name="vp", bufs=1))
        yp = ctx.enter_context(tc.tile_pool(name="yp", bufs=1))
        ep = ctx.enter_context(tc.tile_pool(name="ep", bufs=4))
        rp = ctx.enter_context(tc.tile_pool(name="rp", bufs=2))
        rbp = ctx.enter_context(tc.tile_pool(name="rbp", bufs=2))
        wpp = ctx.enter_context(tc.tile_pool(name="wpp", bufs=1))
        yfp = ctx.enter_context(tc.tile_pool(name="yfp", bufs=2))
        osb = ctx.enter_context(tc.tile_pool(name="osb", bufs=3))
        dram = ctx.enter_context(tc.tile_pool(name="dram", bufs=1, space="DRAM"))

        # ---- persistent SBUF tensors + loads -------------------------------
        Wqk = [const.tile([128, 2 * CH], F32R, name=f"wqk{k}") for k in range(KT)]
        Wv = [const.tile([128, CH], F32R, name=f"wv{k}") for k in range(KT)]
        bqk = const.tile([128, 4], F32, name="bqk")
        bvrow = const.tile([1, CH], F32, name="bvrow")
        bvbc = const.tile([128, CH], F32, name="bvbc")
        ones4 = const.tile([128, 4], F32, name="ones4")
        nc.vector.memset(ones4[:], 1.0)

        for k in range(KT):
            nc.sync.dma_start(Wqk[k][:], w_qk[128 * k:128 * k + 128, :])
        nc.sync.dma_start(bqk[:], b_qk.rearrange("(i p) -> p i", p=128))
        nc.sync.dma_start(bvrow[:], b_v.unsqueeze(0))
        nc.gpsimd.partition_broadcast(bvbc[:], bvrow[:])

        # W_proj (padded to 2048 rows) -> bf16 tiles, converted on device.
        # (tiles declared here; DMAs emitted after the x loads below so the
        # first QKV matmuls aren't starved behind 2MB of projection weights)
        Wp = [wpp.tile([128, CH], BF16, name=f"wp{k}") for k in range(KT2)]
        wpf = [wpp.tile([128, CH], F32, name=f"wpf{k}") for k in range(KT2)]

        # qk^T tiles (bf16): [o-tile i][t-chunk j] -> [128, 512]
        # i = 0,1: q rows (pre-scaled by 1/sqrt(D) on host); i = 2,3: k rows
        qkT = [[qkp.tile([128, 512], BF16, name=f"qkT{i}_{j}") for j in range(NTC)]
               for i in range(4)]
        # v tiles (bf16), head-interleaved with a ones column: [128, 4*65]
        v_sb = [vp.tile([128, HPC * (D + 1)], BF16, name=f"v{m}") for m in range(NTT)]
        # normalized y^T chunk tiles (bf16): [chunk j] -> [256, 512] as 2x128
        yT = [[yp.tile([128, 512], BF16, name=f"yT{i}_{j}") for j in range(NTC)]
              for i in range(2)]

        # AllGather bounce buffers, one pair per chunk
        cc_in = [dram.tile([CH, 512], BF16, name=f"cc_in{j}") for j in range(NTC)]
        cc_out = [dram.tile([NCORES * CH, 512], BF16, name=f"cc_out{j}",
                            addr_space="Shared")
                  for j in range(NTC)]

        with tc.tile_pool(name="xp", bufs=1) as xp, \
             tc.tile_pool(name="psA", bufs=3, space="PSUM") as psA, \
             tc.tile_pool(name="psB", bufs=2, space="PSUM") as psB:
            # x^T loaded in 512-column chunks so phase A starts early
            xT = [[xp.tile([128, 512], F32R, name=f"xT{k}_{j}") for j in range(NTC)]
                  for k in range(KT)]
            for j in range(NTC):
                for k in range(KT):
                    nc.sync.dma_start(xT[k][j][:],
                                      x_t[128 * k:128 * k + 128,
                                          512 * j:512 * j + 512])
            for k in range(KT):
                nc.sync.dma_start(Wv[k][:], w_v[128 * k:128 * k + 128, :])
            for k in range(KT2):
                nc.sync.dma_start(wpf[k][:], w_p[128 * k:128 * k + 128, :])
                nc.vector.tensor_copy(Wp[k][:], wpf[k][:])

            # ---- phase A: qk^T = W_qk^T @ x^T ------------------------------
            for j in range(NTC):
                for i in range(4):
                    ps = psA.tile([128, 512], F32, name="psA", tag="psA")
                    for k in range(KT):
                        nc.tensor.matmul(
                            ps[:],
                            Wqk[k][:, 128 * i:128 * i + 128],
                            xT[k][j][:],
                            start=(k == 0), stop=(k == KT - 1))
                    nc.vector.tensor_scalar_add(qkT[i][j][:], ps[:], bqk[:, i:i + 1])

            # ---- phase B: v = x @ W_v (natural layout, +bias, interleave) --
            for m in range(NTT):
                ones_ap = v_sb[m].rearrange("p (h x) -> p h x", x=D + 1)[:, :, D:D + 1]
                nc.vector.tensor_copy(ones_ap, ones4.rearrange("p (h x) -> p h x", x=1))
                ps = psB.tile([128, CH], F32, name="psB", tag="psB")
                for k in range(KT):
                    nc.tensor.matmul(
                        ps[:],
                        xT[k][m // 4][:, 128 * (m % 4):128 * (m % 4) + 128],
                        Wv[k][:],
                        start=(k == 0), stop=(k == KT - 1))
                v_ap = v_sb[m].rearrange("p (h x) -> p h x", x=D + 1)[:, :, 0:D]
                nc.vector.tensor_add(
                    v_ap,
                    ps.rearrange("p (h d) -> p h d", d=D),
                    bvbc.rearrange("p (h d) -> p h d", d=D))

        # ---- phases C/D/E: chunk-major attention + pipelined AG + proj -----
        # Heads are processed in pairs (2hp, 2hp+1). The even head's q/k rows
        # live at partitions 0-63, the odd head's at 64-127, so interleaving
        # their S matmuls alternates PE row groups (tile_position auto-derives
        # from base_partition): the next weight load overlaps the in-flight
        # matmul and the two K=64 matmuls stream concurrently.
        def attn_chunk(j):
            for hp in range(HPC // 2):
                ha, hb = 2 * hp, 2 * hp + 1
                y_psA = psY.tile([D + 1, 512], F32, name="y_psA", tag="y_ps")
                y_psB = psY.tile([D + 1, 512], F32, name="y_psB", tag="y_ps")
                n_s = 4 * (j + 1)           # causal s-tiles for this chunk
                for sp in range(n_s // 2):  # pairs of 128-row s-tiles
                    sA = psS.tile([128, 1024], F32, name="sA", tag="s_ps")
                    sB = psS.tile([128, 1024], F32, name="sB", tag="s_ps")
                    eA = ep.tile([128, 1024], BF16, name="eA", tag="e_sb")
                    eB = ep.tile([128, 1024], BF16, name="eB", tag="e_sb")
                    for half in range(2):
                        st = 2 * sp + half
                        kt = qkT[2 + hp][st // 4]
                        qt = qkT[hp][j]
                        ks = 128 * (st % 4)
                        nc.tensor.matmul(
                            sA[:, 512 * half:512 * half + 512],
                            kt[0:64, ks:ks + 128], qt[0:64, :],
                            start=True, stop=True)
                        nc.tensor.matmul(
                            sB[:, 512 * half:512 * half + 512],
                            kt[64:128, ks:ks + 128], qt[64:128, :],
                            start=True, stop=True)
                    nc.scalar.activation(
                        eA[:], sA[:], mybir.ActivationFunctionType.Exp)
                    nc.scalar.activation(
                        eB[:], sB[:], mybir.ActivationFunctionType.Exp)
                    if 2 * sp >= 4 * j:     # pair straddles the diagonal
                        r_idx = (2 * sp - 4 * j) // 2
                        for e in (eA, eB):
                            nc.gpsimd.affine_select(
                                out=e[:], in_=e[:],
                                compare_op=mybir.AluOpType.is_ge,
                                fill=0.0,
                                base=-256 * r_idx,
                                pattern=[[-128, 2], [1, 512]],
                                channel_multiplier=-1)
                    for half in range(2):
                        st = 2 * sp + half
                        nc.tensor.matmul(
                            y_psA[:],
                            v_sb[st][:, (D + 1) * ha:(D + 1) * ha + D + 1],
                            eA[:, 512 * half:512 * half + 512],
                            start=(st == 0), stop=(st == n_s - 1))
                        nc.tensor.matmul(
                            y_psB[:],
                            v_sb[st][:, (D + 1) * hb:(D + 1) * hb + D + 1],
                            eB[:, 512 * half:512 * half + 512],
                            start=(st == 0), stop=(st == n_s - 1))
                # normalize: y * (1/rowsum); broadcast the raw rowsum across
                # partitions first so the reciprocal runs at full width
                for hh, y_ps in ((ha, y_psA), (hb, y_psB)):
                    r_sb = rp.tile([1, 512], F32, name="r_sb", tag="r_sb")
                    nc.vector.tensor_copy(r_sb[:], y_ps[D:D + 1, :])
                    rbc = rbp.tile([D, 512], F32, name="rbc", tag="rbc")
                    rinv = rbp.tile([D, 512], F32, name="rinv", tag="rinv")
                    nc.gpsimd.partition_broadcast(rbc[:], r_sb[:])
                    nc.vector.reciprocal_approx_fast(rinv[:], rbc[:])
                    nc.vector.tensor_mul(
                        yT[hp][j][64 * (hh % 2):64 * (hh % 2) + 64, :],
                        y_ps[0:D, :], rinv[:])

        def ship_chunk(j):
            for i in range(2):
                nc.gpsimd.dma_start(cc_in[j][128 * i:128 * i + 128, :],
                                    yT[i][j][:])
            nc.gpsimd.collective_compute(
                "AllGather", mybir.AluOpType.bypass,
                replica_groups=RG8,
                ins=[cc_in[j].opt()], outs=[cc_out[j].opt()])

        def proj_chunk(j):
            yf = [yfp.tile([128, 512], BF16, name=f"yf{k}", tag=f"yf{k}")
                  for k in range(KT2)]
            for k in range(KT2):
                nc.sync.dma_start(yf[k][:],
                                  cc_out[j][128 * k:128 * k + 128, :])
            for m in range(4):
                o_sb = osb.tile([128, CH], F32, name="o_sb", tag="o_sb")
                ps = psE.tile([128, CH], F32, name="psE", tag="psE")
                for k in range(KT2):
                    nc.tensor.matmul(
                        ps[:],
                        yf[k][:, 128 * m:128 * m + 128],
                        Wp[k][:],
                        start=(k == 0), stop=(k == KT2 - 1))
                nc.vector.tensor_copy(o_sb[:], ps[:])
                nc.scalar.dma_start(
                    out[512 * j + 128 * m:512 * j + 128 * m + 128, :],
                    o_sb[:])

        # Skew projection one chunk behind attention: PE executes in issue
        # order, so proj(j) is only enqueued after attn(j+1) - by then the
        # chunk-j AllGather has completed and the PE never stalls on it.
        with tc.tile_pool(name="psS", bufs=2, space="PSUM") as psS, \
             tc.tile_pool(name="psY", bufs=2, space="PSUM") as psY, \
             tc.tile_pool(name="psE", bufs=2, space="PSUM") as psE:
            for j in range(NTC):
                attn_chunk(j)
                ship_chunk(j)
                if j > 0:
                    proj_chunk(j - 1)
            proj_chunk(NTC - 1)

    nc.compile()
    return nc


def shard_inputs(x, W_attn, b_attn, W_proj, b_proj):
    scale = np.float32(D ** -0.5)
    in_maps = []
    for c in range(NCORES):
        b, g = divmod(c, TP)
        q = slice(CH * g, CH * (g + 1))
        k = slice(C + CH * g, C + CH * (g + 1))
        v = slice(2 * C + CH * g, 2 * C + CH * (g + 1))
        W_qk = np.concatenate([W_attn[:, q] * scale, W_attn[:, k]], axis=1)
        b_qk = np.concatenate([b_attn[q] * scale, b_attn[k]])
        # W_proj column slice, zero-padded to 2048 rows so the contraction
        # over the 8-core-gathered [2048, t] y picks out this core's batch
        w_p_pad = np.zeros((B * C, CH), dtype=np.float32)
        w_p_pad[C * b:C * (b + 1)] = W_proj[:, CH * g:CH * (g + 1)]
        in_maps.append({
            "x_t": np.ascontiguousarray(x[b].T, dtype=np.float32),
            "w_qk": np.ascontiguousarray(W_qk, dtype=np.float32),
            "b_qk": np.ascontiguousarray(b_qk, dtype=np.float32),
            "w_v": np.ascontiguousarray(W_attn[:, v], dtype=np.float32),
            "b_v": np.ascontiguousarray(b_attn[v], dtype=np.float32),
            "w_p": w_p_pad,
        })
    return in_maps


_NC_CACHE = {}


def get_compiled():
    if "nc" not in _NC_CACHE:
        _NC_CACHE["nc"] = build_kernel()
    return _NC_CACHE["nc"]


def run_on_hw(in_maps, **kwargs):
    nc = get_compiled()
    return bass_utils.run_bass_kernel_spmd(
        nc, in_maps, core_ids=list(range(NCORES)), **kwargs)


def kernel(x, W_attn, b_attn, W_proj, b_proj):
    x = np.asarray(x, dtype=np.float32)
    W_attn = np.asarray(W_attn, dtype=np.float32)
    b_attn = np.asarray(b_attn, dtype=np.float32)
    W_proj = np.asarray(W_proj, dtype=np.float32)
    b_proj = np.asarray(b_proj, dtype=np.float32)

    in_maps = shard_inputs(x, W_attn, b_attn, W_proj, b_proj)
    res = run_on_hw(in_maps)
    out = np.zeros((B, T, C), dtype=np.float32)
    for c in range(NCORES):
        b, g = divmod(c, TP)
        out[b, :, CH * g:CH * (g + 1)] = res.results[c]["out"]
    out += b_proj[None, None, :]
    return out
